# revision 28
# baseline (speedup 1.0000x reference)
"""Trainium2 Bass kernel for nn_AttentionCombine (v2: SWDGE dma_gather).

Self-contained: builds an SPMD Bass graph (same graph on 8 NeuronCores),
shards inputs data-parallel over the batch dim (4 images / 256 objects per
core), runs via run_bass_kernel_spmd, and reassembles the full output.

Key idea vs v1: instead of loading the full 13MB feature map into SBUF and
running two ~112us GpSimd ap_gathers, the host pre-builds a [25600, 256]
bf16 DRAM layout per image in which unit (y*2 + x%2)*80 + x//2 holds the
2x2 bilinear corner block for base pixel (y, x): 4 pixels x 64 channels =
512 bytes. One SWDGE dma_gather descriptor per sampled contour point
(2048/image) then pulls exactly the needed corner blocks straight from HBM
and lands them transposed in SBUF as [128 part = (x-corner, channel),
2 rows, 2048 points]. GpSimd only generates descriptors (~1.7us/image);
the 16 DMA engines move ~1MB/image.

Per-core dataflow:
  - 4x dma_gather (one per image) -> G[g] [128, 2, 2048] bf16
  - bilinear weights (host-computed, bf16) partition-broadcast, one
    vector multiply, row-pair add, cross-partition x-corner add
  - K-tile assembly [(channel, point-parity) x objects] (odd-parity half
    moved with a SBUF->SBUF DMA)
  - conv1d == GEMM over K=(66 ch x 32 pts) on TensorE (bf16, fp32 psum)
  - + positional embedding (host-gathered, device add)
  - qk GEMM (attention in_proj, p_w/sqrt(hd) folded into q rows on host)
  - attention: per image, 4 accumulating K=128 matmuls (each contracts a
    128-channel chunk = two heads at once; valid because the per-head
    combine weights are already folded into q)
  - sigmoid on ScalarE, DMA out
"""
import os
import sys

for _p in ("/opt/trn_rl_repo", "/root/.axon_site/_ro/trn_rl_repo"):
    if os.path.isdir(_p) and _p not in sys.path:
        sys.path.append(_p)

import numpy as np
from contextlib import ExitStack

from concourse import bacc, mybir
from concourse.tile import TileContext
from concourse.bass_utils import run_bass_kernel_spmd

F32 = mybir.dt.float32
BF16 = mybir.dt.bfloat16
I16 = mybir.dt.int16

# Problem constants (hardcoded per spec)
B, C, H, W = 32, 64, 160, 160
IMG_HW = 640
N_OBJ = 2048
NUM_POINTS = 128
STRIDE = 4
P = NUM_POINTS // STRIDE  # 32 sampled points
NE = 512                  # n_embd
HEADS = 8
PATCH = 16
T = 64                    # objects per image
N_CORES = 8
IMGS_PER_CORE = B // N_CORES      # 4
OBJS_PER_CORE = N_OBJ // N_CORES  # 256
HW_PIX = H * W                    # 25600
NPT = T * P                       # 2048 point-samples per image
NUNITS = H * 2 * (W // 2)         # 25600 gather units per image

_MODEL_CACHE = {}


def build_model():
    if "nc" in _MODEL_CACHE:
        return _MODEL_CACHE["nc"]
    nc = bacc.Bacc("TRN2", target_bir_lowering=False, debug=False,
                   num_swdge_queues=4)
    AL = mybir.AluOpType
    AF = mybir.ActivationFunctionType

    gsrc_e = nc.declare_dram_parameter("gsrc", [4, NUNITS, 256], BF16, isOutput=False)
    idx_e = nc.declare_dram_parameter("idx", [4, 128, NPT // 16], I16, isOutput=False)
    wgt_e = nc.declare_dram_parameter("wgt", [4, 2, 2 * NPT], BF16, isOutput=False)
    ktn_e = nc.declare_dram_parameter("ktn", [128, 256], BF16, isOutput=False)
    cw_e = nc.declare_dram_parameter("cw", [128, 17 * 4 * 128], BF16, isOutput=False)
    aw_e = nc.declare_dram_parameter("aw", [128, 4 * 8 * 128], BF16, isOutput=False)
    posb_e = nc.declare_dram_parameter("posb", [128, 4 * 256], BF16, isOutput=False)
    ab_e = nc.declare_dram_parameter("ab", [128, 8], F32, isOutput=False)
    out_e = nc.declare_dram_parameter("out", [4, 64, 64], F32, isOutput=True)

    with TileContext(nc) as tc, ExitStack() as ctx:
        idxp = ctx.enter_context(tc.tile_pool(name="idxp", bufs=4))
        gp = ctx.enter_context(tc.tile_pool(name="gp", bufs=4))
        wp = ctx.enter_context(tc.tile_pool(name="wp", bufs=4))
        rp = ctx.enter_context(tc.tile_pool(name="rp", bufs=2))
        sp = ctx.enter_context(tc.tile_pool(name="sp", bufs=2))
        afp = ctx.enter_context(tc.tile_pool(name="afp", bufs=2))
        cfp = ctx.enter_context(tc.tile_pool(name="cfp", bufs=1))
        qkp = ctx.enter_context(tc.tile_pool(name="qkp", bufs=1))
        attp = ctx.enter_context(tc.tile_pool(name="attp", bufs=4))
        psp = ctx.enter_context(tc.tile_pool(name="psp", bufs=6, space="PSUM"))
        psap = ctx.enter_context(tc.tile_pool(name="psap", bufs=2, space="PSUM"))

        # idx tiles first, on the scalar HWDGE queue, so the first desc-gen
        # starts as early as possible (not queued behind megabyte consts)
        IDXs = []
        for g in range(4):
            IDX = idxp.tile([128, NPT // 16], I16, tag="idx")
            nc.sync.dma_start(IDX[:], idx_e[g])
            IDXs.append(IDX)

        # Gather in chunks: the SWDGE ring carveout holds ~128 descriptors
        # per engine; the transpose rx side needs 2*nidx/16+2, so nidx must
        # stay under ~1000 per instruction.
        CHUNKS = [768, 768, 512]
        Gs, Ws = [], []
        for g in range(4):
            G = gp.tile([128, 2 * NPT], BF16, tag="g")
            Gs.append(G)
        # Each image's chunks go to different SWDGE queues (queue q -> Q7
        # core pair q), so desc-gen runs on multiple core pairs at once and
        # each image's gather completes early. queue = emission index % 4
        # keeps Tile's rotating DMASW sem lanes aligned with their locked
        # queue (lane l serves queue l%4).
        # Chunk-major emission, queue g fixed per image: desc-gen for the
        # four images overlaps on separate core pairs, each image's chunks
        # drain sequentially through its own ring. (Spreading one image's
        # chunks across queues corrupts data on HW — do not reorder.)
        nreg = {sz: nc.gpsimd.to_reg(sz) for sz in sorted(set(CHUNKS))}
        off = 0
        for sz in CHUNKS:
            for g in range(4):
                Gc = Gs[g][:, 2 * off:2 * (off + sz)].rearrange(
                    "p (j n) -> p j n", j=2, n=sz)
                with nc.named_scope(f"gather_{g}"):
                    nc.gpsimd.dma_gather(
                        Gc, gsrc_e[g],
                        IDXs[g][:, off // 16:(off + sz) // 16],
                        sz, nreg[sz], 256, transpose=True, queue_num=g)
            off += sz

        for g in range(4):
            WT = wp.tile([128, 2 * NPT], BF16, tag="w")
            nc.sync.dma_start(WT[0:64, :], wgt_e[g, 0].partition_broadcast(64))
            nc.sync.dma_start(WT[64:128, :], wgt_e[g, 1].partition_broadcast(64))
            Ws.append(WT)

        # consts load after the idx/weight tiles (needed only at GEMM time)
        const = ctx.enter_context(tc.tile_pool(name="const", bufs=1))
        cw_sb = const.tile([128, 17 * 4 * 128], BF16, tag="cw")
        aw_sb = const.tile([128, 4 * 8 * 128], BF16, tag="aw")
        posb_sb = const.tile([128, 1024], BF16, tag="posb")
        ab_sb = const.tile([128, 8], F32, tag="ab")
        nc.scalar.dma_start(cw_sb[:], cw_e[:])
        nc.scalar.dma_start(aw_sb[:], aw_e[:])
        nc.scalar.dma_start(posb_sb[:], posb_e[:])
        nc.scalar.dma_start(ab_sb[:], ab_e[:])

        ktp = ctx.enter_context(tc.tile_pool(name="kt", bufs=1))
        KT = ktp.tile([128, 17 * 256], BF16, tag="kt")
        nc.scalar.dma_start(KT[:, 16 * 256:17 * 256], ktn_e[:])

        cwv = cw_sb[:].rearrange("p (j o m) -> p j o m", j=17, o=4, m=128)
        awv = aw_sb[:].rearrange("p (k m c) -> p k m c", k=4, m=8, c=128)
        posv = posb_sb[:].rearrange("p (o n) -> p o n", o=4, n=256)
        KTj = KT[:].rearrange("p (j n) -> p j n", j=17, n=256)
        # k-tiles 0..15: row q = (channel q%64, point 2j + q//64); col (g, t)
        KTx = KT[:, 0:16 * 256].rearrange("p (j g t) -> p j g t", j=16, g=4, t=64)

        for g in range(4):
            G, WT = Gs[g], Ws[g]
            with nc.named_scope(f"comb_{g}"):
                # bilinear weights (free layout (c, j, n) on both sides)
                nc.vector.tensor_tensor(G[:], G[:], WT[:], AL.mult)
                # row-pair reduction -> R [128, NPT] (free order (c, n) == i)
                R = rp.tile([128, NPT], BF16, tag="r")
                off = 0
                for sz in CHUNKS:
                    Gc = G[:, 2 * off:2 * (off + sz)].rearrange(
                        "p (j n) -> p j n", j=2, n=sz)
                    nc.vector.tensor_tensor(R[:, off:off + sz],
                                            Gc[:, 0, :], Gc[:, 1, :], AL.add)
                    off += sz
                # x-corner reduction: stage upper half to lower partitions
                # (scalar HWDGE queue: uncongested once consts are in)
                ST = sp.tile([128, NPT], BF16, tag="st")
                nc.scalar.dma_start(ST[0:64, :], R[64:128, :])
                # F free layout: (pt 32, t 64); even/odd point parity split
                Rv = R[:].rearrange("p (a s t) -> p a s t", a=16, s=2, t=64)
                Sv = ST[:].rearrange("p (a s t) -> p a s t", a=16, s=2, t=64)
                # even parity -> KT lower rows directly
                nc.vector.tensor_tensor(KTx[0:64, :, g, :], Rv[0:64, :, 0, :],
                                        Sv[0:64, :, 0, :], AL.add)
                # odd parity -> staging, then SBUF->SBUF DMA to upper rows
                AF_ = afp.tile([128, NPT // 2], BF16, tag="af")
                Av = AF_[:].rearrange("p (a t) -> p a t", a=16, t=64)
                nc.vector.tensor_tensor(Av[0:64], Rv[0:64, :, 1, :],
                                        Sv[0:64, :, 1, :], AL.add)
                nc.scalar.dma_start(KTx[64:128, :, g, :], Av[0:64])

        # GEMM1 (conv) per image pair (N=128 object-columns)
        CF = cfp.tile([128, 4, 256], BF16, tag="cf")
        QK = qkp.tile([128, 8, 256], BF16, tag="qk")
        for pp in range(2):
            nsl = slice(pp * 128, pp * 128 + 128)
            with nc.named_scope(f"gemm1_{pp}"):
                for o in range(4):
                    ps = psp.tile([128, 128], F32, tag="ps")
                    for j in range(17):
                        nc.tensor.matmul(ps[:], lhsT=cwv[:, j, o, :],
                                         rhs=KTj[:, j, nsl],
                                         start=(j == 0), stop=(j == 16))
                    nc.vector.tensor_tensor(CF[:, o, nsl], ps[:],
                                            posv[:, o, nsl], AL.add)

            # GEMM2 (attention in_proj)
            with nc.named_scope(f"gemm2_{pp}"):
                for m8 in range(8):
                    ps = psp.tile([128, 128], F32, tag="ps")
                    for k in range(4):
                        nc.tensor.matmul(ps[:], lhsT=awv[:, k, m8, :],
                                         rhs=CF[:, k, nsl],
                                         start=(k == 0), stop=(k == 3))
                    nc.scalar.activation(QK[:, m8, nsl], ps[:],
                                         AF.Identity, bias=ab_sb[:, m8:m8 + 1])

            # attention per image: att[t, s] = sum_h (p_w[h]/8) Q_h K_h^T
            for hh in range(2):
                g = 2 * pp + hh
                tsl = slice(g * 64, g * 64 + 64)
                ps = psap.tile([64, 64], F32, tag="psa")
                for qc in range(4):
                    nc.tensor.matmul(ps[:], lhsT=QK[:, qc, tsl],
                                     rhs=QK[:, 4 + qc, tsl],
                                     start=(qc == 0), stop=(qc == 3))
                ATT = attp.tile([64, 64], F32, tag="att")
                nc.scalar.activation(ATT[:], ps[:], AF.Sigmoid)
                nc.sync.dma_start(out_e[g], ATT[:])

    nc.compile()
    _MODEL_CACHE["nc"] = nc
    return nc


def host_prep(inputs):
    """Host-side sharding + layout prep. Returns list of 8 per-core input maps."""
    import ml_dtypes
    bf = ml_dtypes.bfloat16

    cnn = np.ascontiguousarray(np.asarray(inputs["cnn_feature"], dtype=np.float32))
    contours = np.asarray(inputs["contours"], dtype=np.float32)
    ct_01 = np.asarray(inputs["ct_01"])
    ct_img_idx = np.asarray(inputs["ct_img_idx"])
    ct_ind = np.asarray(inputs["ct_ind"])
    h = int(inputs["h"]); w = int(inputs["w"])
    conv_w = np.asarray(inputs["conv_w"], dtype=np.float32)
    conv_b = np.asarray(inputs["conv_b"], dtype=np.float32)
    attn_w = np.asarray(inputs["attn_w"], dtype=np.float32)
    attn_b = np.asarray(inputs["attn_b"], dtype=np.float32)
    p_w = np.asarray(inputs["p_w"], dtype=np.float32)
    pos_embed = np.asarray(inputs["pos_embed"], dtype=np.float32)

    assert bool(np.all(ct_01)), "kernel requires ct_01 all ones"
    assert bool(np.all(ct_img_idx == np.repeat(np.arange(B, dtype=ct_img_idx.dtype), T)))

    # ---- gather source layout: per image, unit (y*2 + x%2)*80 + x//2 holds
    # the 2x2 corner block for base (y, x): [sy, sx, ch] bf16 = 512B.
    cnn16 = cnn.astype(bf).view(np.uint16)                    # [B, C, H, W]
    Tm = np.ascontiguousarray(cnn16.transpose(0, 2, 3, 1))    # [B, H, W, C]
    Tp = np.zeros((B, H + 1, W + 1, C), np.uint16)
    Tp[:, :H, :W] = Tm
    U = np.empty((B, H, 2, W // 2, 4, C), np.uint16)
    for par in range(2):
        s0 = slice(par, par + W, 2)
        s1 = slice(par + 1, par + 1 + W, 2)
        U[:, :, par, :, 0] = Tp[:, 0:H, s0]
        U[:, :, par, :, 1] = Tp[:, 0:H, s1]
        U[:, :, par, :, 2] = Tp[:, 1:H + 1, s0]
        U[:, :, par, :, 3] = Tp[:, 1:H + 1, s1]
    U = U.reshape(B, NUNITS, 256).view(bf)

    # ---- bilinear base coords, unit indices, corner weights
    cs = np.ascontiguousarray(contours[:, ::STRIDE])          # [N, 32, 2]
    px = cs[..., 0] * (float(W) / w) - 0.5
    py = cs[..., 1] * (float(H) / h) - 0.5
    x0 = np.floor(px); y0 = np.floor(py)
    wx1 = px - x0; wx0 = 1.0 - wx1
    wy1 = py - y0; wy0 = 1.0 - wy1
    xb = np.clip(x0, 0, W - 2).astype(np.int32)
    yb = np.clip(y0, 0, H - 2).astype(np.int32)
    unit = (yb * 2 + (xb & 1)) * (W // 2) + (xb >> 1)         # [N, 32]

    def side_w(base, c0, w0c, w1c, lim):
        # weight of corner at coordinate base+s for s in (0, 1)
        out = []
        for s in range(2):
            d = base + s - c0
            val = np.where(d == 0, w0c * ((c0 >= 0) & (c0 < lim)),
                           np.where(d == 1, w1c * ((c0 + 1) >= 0) * ((c0 + 1) < lim),
                                    0.0))
            out.append(val.astype(np.float32))
        return out

    wxs = side_w(xb, x0, wx0, wx1, W)                         # [2][N, 32]
    wys = side_w(yb, y0, wy0, wy1, H)
    # corner weight [sy][sx] = wys[sy] * wxs[sx]
    wc = [[wys[sy] * wxs[sx] for sx in range(2)] for sy in range(2)]

    normed = cs / np.array([w, h], np.float32)                # [N, 32, 2]

    ct_x = (ct_ind % W).astype(np.int64) * PATCH // W
    ct_y = (ct_ind // W).astype(np.int64) * PATCH // H
    posb_full = pos_embed[:, ct_y, ct_x] + conv_b[:, None]    # [512, N]

    s = np.ones(2 * NE, np.float32)
    s[:NE] = np.repeat(p_w[0, :, 0], NE // HEADS) / np.sqrt(np.float32(NE // HEADS))
    aw_t = (attn_w * s[:, None]).T                            # [512, 1024] (k, m)
    ab = attn_b * s                                           # [1024]

    # conv_w K-tiles -> cwT [128, 17*4*128]
    cw = np.zeros((17, 128, 512), np.float32)
    q = np.arange(128)
    for j in range(16):
        cw[j] = conv_w[:, q % 64, 2 * j + q // 64].T          # [128, 512]
    q64 = np.arange(64)
    cw[16, :64] = conv_w[:, 64 + q64 // 32, q64 % 32].T
    cwT = cw.reshape(17, 128, 4, 128).transpose(1, 0, 2, 3).reshape(128, 17 * 4 * 128)

    awT = aw_t.reshape(4, 128, 8, 128).transpose(1, 0, 2, 3).reshape(128, 4 * 8 * 128)
    abT = np.ascontiguousarray(ab.reshape(8, 128).T)          # [128, 8]

    # per-image point-sample order i = pt*64 + t
    unit_img = unit.reshape(B, T, P).transpose(0, 2, 1).reshape(B, NPT)
    wc_img = [[wc[sy][sx].reshape(B, T, P).transpose(0, 2, 1).reshape(B, NPT)
               for sx in range(2)] for sy in range(2)]

    # gathers run in chunks (sizes CHUNKS, matching the device); chunk at
    # offset `off` covers i in [off, off+sz). Wrapped idx [128, NPT//16]:
    # col off//16+f, partition p -> i = off + f*16 + p%16; tiled x8.
    CHUNKS = [768, 768, 512]
    idx_w16 = np.empty((B, 16, NPT // 16), np.int16)
    off = 0
    for sz in CHUNKS:
        blk = unit_img[:, off:off + sz].reshape(B, sz // 16, 16)
        idx_w16[:, :, off // 16:(off + sz) // 16] = blk.transpose(0, 2, 1)
        off += sz
    idx_wrapped = np.tile(idx_w16, (1, 8, 1)).astype(np.int16)  # [B, 128, ...]

    # weight rows: wgt[img, sx, (ch, sy, isub)] matching G free layout
    wgt = np.empty((B, 2, 2 * NPT), np.float32)
    off = 0
    for sz in CHUNKS:
        for sy in range(2):
            for sx in range(2):
                wgt[:, sx, 2 * off + sy * sz:2 * off + (sy + 1) * sz] = \
                    wc_img[sy][sx][:, off:off + sz]
        off += sz

    in_maps = []
    for core in range(N_CORES):
        imgs = slice(IMGS_PER_CORE * core, IMGS_PER_CORE * (core + 1))
        nbase = OBJS_PER_CORE * core

        # ktnorm [128, 256]: q<64: (coord=q//32, p=q%32); cols (g, t)
        ktn = np.zeros((128, 256), np.float32)
        ncols = nbase + np.arange(256)
        ktn[:64] = normed[ncols][:, np.arange(64) % 32, np.arange(64) // 32].T

        posbT = np.ascontiguousarray(
            posb_full[:, nbase:nbase + 256].reshape(4, 128, 256)
            .transpose(1, 0, 2).reshape(128, 1024))

        in_maps.append({
            "gsrc": U[imgs],
            "idx": idx_wrapped[imgs],
            "wgt": wgt[imgs].astype(bf),
            "ktn": ktn.astype(bf),
            "cw": cwT.astype(bf),
            "aw": awT.astype(bf),
            "posb": posbT.astype(bf),
            "ab": abT.astype(np.float32),
        })
    return in_maps


def run(in_maps, trace=False, **kw):
    nc = build_model()
    res = run_bass_kernel_spmd(nc, in_maps, core_ids=list(range(N_CORES)),
                               trace=trace, **kw)
    return res


def kernel(**inputs):
    in_maps = host_prep(inputs)
    res = run(in_maps)
    out = np.concatenate([res.results[i]["out"] for i in range(N_CORES)], axis=0)
    return out.astype(np.float32)
